# revision 4
# baseline (speedup 1.0000x reference)
"""Trainium2 Bass kernel for nn_DynamicDictionaryLearning (vq_codebook).

Computation (full shapes):
    query_embed = (basic_queries @ W_mlp + b_mlp).reshape(T, R, D)    # (T, R*D)
    dynamic_queries = einsum('btr,trd->btd', query_weights, query_embed)
    basic_expanded  = broadcast(basic_queries, (B, T, D))

Sharding (8 NeuronCores, one chip):
    Stage 1 (token-MLP expansion) is tensor-sharded over the R*D output dim:
    core r computes qe_r = basic_queries @ W_mlp[:, r*D:(r+1)*D] + b_r for
    ALL tokens.  An on-chip AllToAll per 256-col chunk redistributes qe so
    core c holds all R slices for its 128-token slice.  Stage 2 (weighted
    sum over R) runs as dense PE matmuls with block-diagonal qw tiles
    (contraction packs r x 16 tokens = 128, output packs 8 batch x 16
    tokens = 128).

    Schedule: stage 1 runs as 512-col k-outer passes (8 PSUM chains, one
    per token tile) so the first pass's PE consumption (~2.1us per k-tile)
    paces with the bq/W k-tile loads.  Each 256-col chunk's AllToAll is
    issued as soon as its slice is stored; stage-2 chunks are interleaved
    into the PE stream two-plus chunks behind so the PE never idles, the
    collectives pipeline, and the dq output stream spreads over the whole
    kernel.  bq loads + ain/dq stores ride the SP HWDGE ring; W/bias/Lt
    loads + q2 gathers ride the ACT ring.  PSUM->SBUF casts alternate
    DVE/ACT.  A dependency-free warmup collective absorbs the CC-pipe
    bring-up cost.  All matmul operands and the dq store are bf16 (fp32
    PSUM accumulation).

    basic_expanded is a pure broadcast of an input -> host-side view.
"""

import numpy as np
import ml_dtypes

import concourse.bass as bass
import concourse.mybir as mybir
import concourse.tile as tile
from concourse import bacc
from concourse.bass_utils import run_bass_kernel_spmd

# Problem shapes (hardcoded per spec)
D = 2048
T = 1024
R = 8
B = 32
NC = 8
TS = T // NC          # 128 tokens per core (stage-2 ownership)
P = 128
KT = D // P           # 16 contraction tiles
MT = T // P           # 8 token tiles (stage 1)
NQ = 8                # a2a chunks (D-slices)
NW = D // NQ          # 256 cols per chunk
TG = TS // 16         # 8 token groups of 16 (stage 2)
BG = B // 8           # 4 batch groups of 8 (stage 2)
PW = 512              # stage-1 pass width (8 PSUM banks)

F32 = mybir.dt.float32
BF16 = mybir.dt.bfloat16
DT_MM = BF16
NP_MM = ml_dtypes.bfloat16

# stage-1 passes: (col_lo, width).  First three are 512 wide; the last 512
# cols split into two 256 passes so chunks 6 and 7 complete staggered and
# their AllToAlls don't serialize at the tail.
PASSES = [(0, 512), (512, 512), (1024, 512), (1536, 256), (1792, 256)]
# s2 chunks emitted after each pass index (PE program order)
S2_AFTER = {1: [0, 1], 2: [2, 3], 3: [4], 4: [5, 6, 7]}

_cache = {}


def _build_nc():
    nc = bacc.Bacc("TRN2", target_bir_lowering=False, num_devices=NC)

    bqT = nc.dram_tensor("bqT", [D, T], DT_MM, kind="ExternalInput")
    Wc = nc.dram_tensor("Wc", [D, D], DT_MM, kind="ExternalInput")
    biasr = nc.dram_tensor("biasr", [P, D], F32, kind="ExternalInput")
    # block-diagonal qw tiles, packed (128, 32*128) for one big-line DMA
    Lt = nc.dram_tensor("Lt", [P, TG * BG * P], DT_MM, kind="ExternalInput")
    # per-(chunk, token-group) contiguous blocks; host reassembles
    dq = nc.dram_tensor("dq", [NQ, TG, P, BG * NW], BF16,
                        kind="ExternalOutput")

    bqT_t = bqT.rearrange("(kt p) m -> kt p m", p=P)   # (16, 128, 1024)
    Wc_t = Wc.rearrange("(kt p) d -> kt p d", p=P)     # (16, 128, 2048)

    with tile.TileContext(nc) as tc:
        with (
            tc.tile_pool(name="bqp", bufs=1) as bqpool,
            tc.tile_pool(name="wp", bufs=1) as wpool,
            tc.tile_pool(name="constp", bufs=1) as cpool,
            tc.tile_pool(name="qep", bufs=2) as qepool,
            tc.tile_pool(name="q2p", bufs=3) as q2pool,
            tc.tile_pool(name="o2p", bufs=4) as o2pool,
            tc.tile_pool(name="psp", bufs=8, space="PSUM") as pspool,
            tc.tile_pool(name="dramp", bufs=1, space="DRAM") as dram,
        ):
            # warmup collective: no data deps, absorbs CC bring-up
            wdum = dram.tile([NC, 16], DT_MM, name="wdum")
            adum = dram.tile([NC, 16], DT_MM, name="adum")
            nc.gpsimd.collective_compute(
                "AllToAll",
                mybir.AluOpType.bypass,
                replica_groups=[list(range(NC))],
                ins=[wdum.opt()],
                outs=[adum.opt()],
            )

            # loads: bq k-tiles on SP ring, W k-tiles on ACT ring, so the
            # first stage-1 pass starts at ~4us and k-tiles arrive paced.
            bq_tiles = []
            w_tiles = []
            for k in range(KT):
                bt = bqpool.tile([P, T], DT_MM, name=f"bq{k}")
                nc.sync.dma_start(out=bt, in_=bqT_t[k])
                bq_tiles.append(bt)
                wt = wpool.tile([P, D], DT_MM, name=f"w{k}")
                nc.scalar.dma_start(out=wt, in_=Wc_t[k])
                w_tiles.append(wt)
            bias_t = cpool.tile([P, D], F32, name="bias")
            nc.scalar.dma_start(out=bias_t, in_=biasr[:, :])
            lbig = cpool.tile([P, TG * BG * P], DT_MM, name="lbig")
            nc.scalar.dma_start(out=lbig, in_=Lt[:, :])
            l_tiles = {
                (g, h): lbig[:, (g * BG + h) * P:(g * BG + h + 1) * P]
                for g in range(TG)
                for h in range(BG)
            }

            ain = [dram.tile([T, NW], DT_MM, name=f"ain{j}") for j in range(NQ)]
            aout = [dram.tile([T, NW], DT_MM, name=f"aout{j}") for j in range(NQ)]

            def a2a(j):
                nc.gpsimd.collective_compute(
                    "AllToAll",
                    mybir.AluOpType.bypass,
                    replica_groups=[list(range(NC))],
                    ins=[ain[j].opt()],
                    outs=[aout[j].opt()],
                )

            def stage1_pass(pi):
                lo, w = PASSES[pi]
                chunks = range(lo // NW, (lo + w) // NW)
                with nc.named_scope(f"s1_p{pi}"):
                    qe = qepool.tile([P, MT * w], DT_MM, name="qe")
                    ps = [pspool.tile([P, PW], F32, name="ps")
                          for _ in range(MT)]
                    for k in range(KT):
                        for m in range(MT):
                            nc.tensor.matmul(
                                ps[m][:, :w],
                                bq_tiles[k][:, m * P:(m + 1) * P],
                                w_tiles[k][:, lo:lo + w],
                                start=(k == 0),
                                stop=(k == KT - 1),
                            )
                    for m in range(MT):
                        nc.vector.tensor_add(
                            qe[:, m * w:(m + 1) * w],
                            ps[m][:, :w],
                            bias_t[:, lo:lo + w],
                        )
                    # per-(chunk, m) stores: SBUF src stays plain 2D
                    for j in chunks:
                        off = j * NW - lo
                        for m in range(MT):
                            nc.sync.dma_start(
                                out=ain[j][m * P:(m + 1) * P, :],
                                in_=qe[:, m * w + off:m * w + off + NW],
                            )
                    for j in chunks:
                        a2a(j)

            def stage2(j):
                with nc.named_scope(f"s2_q{j}"):
                    # one flat gather per chunk: bq tokens are host-side
                    # swizzled (p' = tt*8+g within each 128-token tile) so
                    # aout row r*128 + tt*8 + g maps 1:1 onto q2 partition
                    # (r*16+tt), col (g*NW+c) in plain flat order.
                    q2 = q2pool.tile([P, TG * NW], DT_MM, name="q2")
                    nc.scalar.dma_start(out=q2[:, :], in_=aout[j][:, :])
                    for g in range(TG):
                        o2 = o2pool.tile([P, BG * NW], BF16, name="o2")
                        for h in range(BG):
                            ps2 = pspool.tile([P, PW], F32, name="ps")
                            nc.tensor.matmul(
                                ps2[:, :NW],
                                l_tiles[(g, h)][:, :],
                                q2[:, g * NW:(g + 1) * NW],
                                start=True,
                                stop=True,
                            )
                            dst = o2[:, h * NW:(h + 1) * NW]
                            if h % 2 == 0:
                                nc.vector.tensor_copy(dst, ps2[:, :NW])
                            else:
                                nc.scalar.copy(dst, ps2[:, :NW])
                        # contiguous (128, BG*NW) block store
                        nc.sync.dma_start(out=dq[j, g], in_=o2[:, :])

            for pi in range(len(PASSES)):
                stage1_pass(pi)
                for j in S2_AFTER.get(pi, []):
                    stage2(j)

    nc.finalize()
    return nc


def _prep_inputs(query_weights, basic_queries, W_mlp, b_mlp):
    qw = np.ascontiguousarray(query_weights, dtype=np.float32)
    bq = np.ascontiguousarray(basic_queries, dtype=np.float32)
    W = np.ascontiguousarray(W_mlp, dtype=np.float32)
    b = np.ascontiguousarray(b_mlp, dtype=np.float32)

    # token swizzle: within each 128-token tile, column p' = tt*8+g holds
    # token g*16+tt, so stage-1 output partitions land pre-packed for the
    # stage-2 gather (see stage2() in _build_nc).
    m_i = np.arange(MT)[:, None, None]
    tt_j = np.arange(16)[None, :, None]
    g_j = np.arange(TG)[None, None, :]
    tok_idx = (m_i * P + g_j * 16 + tt_j).reshape(-1)  # new-col -> old token
    bqT = np.ascontiguousarray(bq.T[:, tok_idx].astype(NP_MM))  # (D, T)

    g_i = np.arange(TG)[:, None, None, None, None]
    h_i = np.arange(BG)[None, :, None, None, None]
    tt_i = np.arange(16)[None, None, :, None, None]
    r_i = np.arange(R)[None, None, None, :, None]
    bb_i = np.arange(8)[None, None, None, None, :]

    in_maps = []
    for c in range(NC):
        Wc = np.ascontiguousarray(W[:, c * D:(c + 1) * D].astype(NP_MM))
        biasr = np.ascontiguousarray(
            np.broadcast_to(b[c * D:(c + 1) * D], (P, D))
        )
        qw_c = qw[:, c * TS:(c + 1) * TS, :]  # (32, 128, 8)
        # K index r*16+tt (r-major), M index bb*16+tt (b-major)
        L = np.zeros((TG, BG, P, P), NP_MM)
        L[g_i, h_i, r_i * 16 + tt_i, bb_i * 16 + tt_i] = \
            qw_c[h_i * 8 + bb_i, g_i * 16 + tt_i, r_i].astype(NP_MM)
        # pack to (128, 32*128): Lbig[p, (g*BG+h)*128 + m] = L[g, h, p, m]
        Lbig = np.ascontiguousarray(
            L.transpose(2, 0, 1, 3).reshape(P, TG * BG * P)
        )
        in_maps.append({"bqT": bqT, "Wc": Wc, "biasr": biasr, "Lt": Lbig})
    return in_maps


last_results = None  # exposed for external profiling harnesses


def kernel(query_weights, basic_queries, W_mlp, b_mlp):
    global last_results
    if "nc" not in _cache:
        _cache["nc"] = _build_nc()
    nc = _cache["nc"]

    in_maps = _prep_inputs(query_weights, basic_queries, W_mlp, b_mlp)
    res = run_bass_kernel_spmd(nc, in_maps, core_ids=list(range(NC)))
    last_results = res

    # dq[j, g, (bb,tt), (h,c)] -> (B, TS, D):  b = h*8+bb, t = g*16+tt,
    # d = j*NW+c
    parts = []
    for c in range(NC):
        arr = res.results[c]["dq"].reshape(NQ, TG, 8, 16, BG, NW)
        arr = arr.transpose(4, 2, 1, 3, 0, 5).reshape(B, TS, D)
        parts.append(arr.astype(np.float32))
    dq_full = np.concatenate(parts, axis=1)
    basic_expanded = np.broadcast_to(
        np.ascontiguousarray(basic_queries, dtype=np.float32)[None], (B, T, D)
    )
    return dq_full, basic_expanded


# revision 7
# speedup vs baseline: 1.4907x; 1.4907x over previous
"""Trainium2 Bass kernel for nn_DynamicDictionaryLearning (vq_codebook).

Computation (full shapes):
    query_embed = (basic_queries @ W_mlp + b_mlp).reshape(T, R, D)    # (T, R*D)
    dynamic_queries = einsum('btr,trd->btd', query_weights, query_embed)
    basic_expanded  = broadcast(basic_queries, (B, T, D))

Sharding (8 NeuronCores, one chip):
    Stage 1 (token-MLP expansion) is tensor-sharded over the R*D output dim:
    core r computes qe_r = basic_queries @ W_mlp[:, r*D:(r+1)*D] + b_r for
    ALL tokens.  An on-chip AllToAll per 256-col chunk redistributes qe so
    core c holds all R slices for its 128-token slice.  Stage 2 (weighted
    sum over R) runs as dense PE matmuls with block-diagonal qw tiles
    (contraction packs r x 16 tokens = 128, output packs 8 batch x 16
    tokens = 128).

    Schedule: stage 1 runs as 512-col k-outer passes (8 PSUM chains, one
    per token tile) so the first pass's PE consumption (~2.1us per k-tile)
    paces with the bq/W k-tile loads.  Each 256-col chunk's AllToAll is
    issued as soon as its slice is stored; stage-2 chunks are interleaved
    into the PE stream two-plus chunks behind so the PE never idles, the
    collectives pipeline, and the dq output stream spreads over the whole
    kernel.  bq loads + ain/dq stores ride the SP HWDGE ring; W/bias/Lt
    loads + q2 gathers ride the ACT ring.  PSUM->SBUF casts alternate
    DVE/ACT.  A dependency-free warmup collective absorbs the CC-pipe
    bring-up cost.  All matmul operands and the dq store are bf16 (fp32
    PSUM accumulation).

    basic_expanded is a pure broadcast of an input -> host-side view.
"""

import numpy as np
import ml_dtypes

import concourse.bass as bass
import concourse.mybir as mybir
import concourse.tile as tile
from concourse import bacc
from concourse.bass_utils import run_bass_kernel_spmd

# Problem shapes (hardcoded per spec)
D = 2048
T = 1024
R = 8
B = 32
NC = 8
TS = T // NC          # 128 tokens per core (stage-2 ownership)
P = 128
KT = D // P           # 16 contraction tiles
MT = T // P           # 8 token tiles (stage 1)
NQ = 8                # a2a chunks (D-slices)
NW = D // NQ          # 256 cols per chunk
TG = TS // 16         # 8 token groups of 16 (stage 2)
BG = B // 8           # 4 batch groups of 8 (stage 2)
PW = 512              # stage-1 pass width (8 PSUM banks)

F32 = mybir.dt.float32
BF16 = mybir.dt.bfloat16
DT_MM = BF16
NP_MM = ml_dtypes.bfloat16

# stage-1 passes: (col_lo, width).  First three are 512 wide; the last 512
# cols split into two 256 passes so chunks 6 and 7 complete staggered and
# their AllToAlls don't serialize at the tail.
PASSES = [(0, 512), (512, 512), (1024, 512), (1536, 256), (1792, 256)]
# s2 chunks emitted after each pass index (PE program order)
S2_AFTER = {1: [0, 1], 2: [2, 3], 3: [4], 4: [5, 6, 7]}

_cache = {}


def _build_nc():
    nc = bacc.Bacc("TRN2", target_bir_lowering=False, num_devices=NC)

    bqT = nc.dram_tensor("bqT", [D, T], DT_MM, kind="ExternalInput")
    Wc = nc.dram_tensor("Wc", [D, D], DT_MM, kind="ExternalInput")
    biasr = nc.dram_tensor("biasr", [P, D], F32, kind="ExternalInput")
    # block-diagonal qw tiles, packed (128, 32*128) for one big-line DMA
    Lt = nc.dram_tensor("Lt", [P, TG * BG * P], DT_MM, kind="ExternalInput")
    # per-(chunk, token-group) contiguous blocks; host reassembles
    dq = nc.dram_tensor("dq", [NQ, TG, P, BG * NW], BF16,
                        kind="ExternalOutput")

    bqT_t = bqT.rearrange("(kt p) m -> kt p m", p=P)   # (16, 128, 1024)
    Wc_t = Wc.rearrange("(kt p) d -> kt p d", p=P)     # (16, 128, 2048)

    with tile.TileContext(nc) as tc:
        with (
            tc.tile_pool(name="bqp", bufs=1) as bqpool,
            tc.tile_pool(name="wp", bufs=1) as wpool,
            tc.tile_pool(name="constp", bufs=1) as cpool,
            tc.tile_pool(name="qep", bufs=2) as qepool,
            tc.tile_pool(name="q2p", bufs=3) as q2pool,
            tc.tile_pool(name="o2p", bufs=4) as o2pool,
            tc.tile_pool(name="psp", bufs=4, space="PSUM") as pspool,
            tc.tile_pool(name="dramp", bufs=1, space="DRAM") as dram,
        ):
            # warmup collective: no data deps, absorbs CC bring-up
            wdum = dram.tile([NC, 16], DT_MM, name="wdum")
            adum = dram.tile([NC, 16], DT_MM, name="adum")
            nc.gpsimd.collective_compute(
                "AllToAll",
                mybir.AluOpType.bypass,
                replica_groups=[list(range(NC))],
                ins=[wdum.opt()],
                outs=[adum.opt()],
            )

            # loads: bq k-tiles on SP ring, W k-tiles on ACT ring, so the
            # first stage-1 pass starts at ~4us and k-tiles arrive paced.
            bq_tiles = []
            w_tiles = []
            for k in range(KT):
                bt = bqpool.tile([P, T], DT_MM, name=f"bq{k}")
                nc.sync.dma_start(out=bt, in_=bqT_t[k])
                bq_tiles.append(bt)
                wt = wpool.tile([P, D], DT_MM, name=f"w{k}")
                nc.scalar.dma_start(out=wt, in_=Wc_t[k])
                w_tiles.append(wt)
            bias_t = cpool.tile([P, D], F32, name="bias")
            nc.scalar.dma_start(out=bias_t, in_=biasr[:, :])
            lbig = cpool.tile([P, TG * BG * P], DT_MM, name="lbig")
            nc.scalar.dma_start(out=lbig, in_=Lt[:, :])
            l_tiles = {
                (g, h): lbig[:, (g * BG + h) * P:(g * BG + h + 1) * P]
                for g in range(TG)
                for h in range(BG)
            }

            ain = [dram.tile([T, NW], DT_MM, name=f"ain{j}") for j in range(NQ)]
            aout = [dram.tile([T, NW], DT_MM, name=f"aout{j}") for j in range(NQ)]

            def a2a(j):
                nc.gpsimd.collective_compute(
                    "AllToAll",
                    mybir.AluOpType.bypass,
                    replica_groups=[list(range(NC))],
                    ins=[ain[j].opt()],
                    outs=[aout[j].opt()],
                )

            PSW = 1024  # PSUM slot width (2 banks); pool = 4 slots = 8 banks

            def stage1_pass(pi):
                lo, w = PASSES[pi]
                chunks = range(lo // NW, (lo + w) // NW)
                with nc.named_scope(f"s1_p{pi}"):
                    qe = qepool.tile([P, MT * w], DT_MM, name="qe")
                    ps = [pspool.tile([P, PSW], F32, name="ps")
                          for _ in range(4)]

                    def chain(m):
                        # one accumulation chain per 2KB PSUM bank:
                        # start=True clears the whole bank's has_written
                        # bits, so chains must never share a bank
                        return ps[m // 2][:, (m % 2) * 512:(m % 2) * 512 + w]

                    for k in range(KT):
                        for m in range(MT):
                            nc.tensor.matmul(
                                chain(m),
                                bq_tiles[k][:, m * P:(m + 1) * P],
                                w_tiles[k][:, lo:lo + w],
                                start=(k == 0),
                                stop=(k == KT - 1),
                            )
                    for m in range(MT):
                        nc.vector.tensor_add(
                            qe[:, m * w:(m + 1) * w],
                            chain(m),
                            bias_t[:, lo:lo + w],
                        )
                    # one store per chunk: 3D SBUF src (p, m, c) matched to
                    # a permuted DRAM view of ain[j] (token-major rows)
                    qe3 = qe[:, :].rearrange("p (m c) -> p m c", m=MT)
                    for j in chunks:
                        off = j * NW - lo
                        nc.sync.dma_start(
                            out=ain[j].rearrange("(m p) c -> p m c", m=MT),
                            in_=qe3[:, :, off:off + NW],
                        )
                    for j in chunks:
                        a2a(j)

            def stage2(j):
                with nc.named_scope(f"s2_q{j}"):
                    # one flat gather per chunk: bq tokens are host-side
                    # swizzled (p' = tt*8+g within each 128-token tile) so
                    # aout row r*128 + tt*8 + g maps 1:1 onto q2 partition
                    # (r*16+tt), col (g*NW+c) in plain flat order.
                    q2 = q2pool.tile([P, TG * NW], DT_MM, name="q2")
                    nc.scalar.dma_start(out=q2[:, :], in_=aout[j][:, :])
                    for g in range(TG):
                        o2 = o2pool.tile([P, BG * NW], BF16, name="o2")
                        # 4 batch-group matmuls accumulate into one 2-bank
                        # PSUM tile -> one wide cast
                        ps2 = pspool.tile([P, PSW], F32, name="ps")
                        for h in range(BG):
                            nc.tensor.matmul(
                                ps2[:, h * NW:(h + 1) * NW],
                                l_tiles[(g, h)][:, :],
                                q2[:, g * NW:(g + 1) * NW],
                                start=True,
                                stop=True,
                            )
                        if g % 2 == 0:
                            nc.vector.tensor_copy(o2[:, :], ps2[:, :])
                        else:
                            nc.scalar.copy(o2[:, :], ps2[:, :])
                        # contiguous (128, BG*NW) block store
                        nc.sync.dma_start(out=dq[j, g], in_=o2[:, :])

            for pi in range(len(PASSES)):
                stage1_pass(pi)
                for j in S2_AFTER.get(pi, []):
                    stage2(j)

    nc.finalize()
    return nc


def _prep_inputs(query_weights, basic_queries, W_mlp, b_mlp):
    qw = np.ascontiguousarray(query_weights, dtype=np.float32)
    bq = np.ascontiguousarray(basic_queries, dtype=np.float32)
    W = np.ascontiguousarray(W_mlp, dtype=np.float32)
    b = np.ascontiguousarray(b_mlp, dtype=np.float32)

    # token swizzle: within each 128-token tile, column p' = tt*8+g holds
    # token g*16+tt, so stage-1 output partitions land pre-packed for the
    # stage-2 gather (see stage2() in _build_nc).
    m_i = np.arange(MT)[:, None, None]
    tt_j = np.arange(16)[None, :, None]
    g_j = np.arange(TG)[None, None, :]
    tok_idx = (m_i * P + g_j * 16 + tt_j).reshape(-1)  # new-col -> old token
    bqT = np.ascontiguousarray(bq.T[:, tok_idx].astype(NP_MM))  # (D, T)

    g_i = np.arange(TG)[:, None, None, None, None]
    h_i = np.arange(BG)[None, :, None, None, None]
    tt_i = np.arange(16)[None, None, :, None, None]
    r_i = np.arange(R)[None, None, None, :, None]
    bb_i = np.arange(8)[None, None, None, None, :]

    in_maps = []
    for c in range(NC):
        Wc = np.ascontiguousarray(W[:, c * D:(c + 1) * D].astype(NP_MM))
        biasr = np.ascontiguousarray(
            np.broadcast_to(b[c * D:(c + 1) * D], (P, D))
        )
        qw_c = qw[:, c * TS:(c + 1) * TS, :]  # (32, 128, 8)
        # K index r*16+tt (r-major), M index bb*16+tt (b-major)
        L = np.zeros((TG, BG, P, P), NP_MM)
        L[g_i, h_i, r_i * 16 + tt_i, bb_i * 16 + tt_i] = \
            qw_c[h_i * 8 + bb_i, g_i * 16 + tt_i, r_i].astype(NP_MM)
        # pack to (128, 32*128): Lbig[p, (g*BG+h)*128 + m] = L[g, h, p, m]
        Lbig = np.ascontiguousarray(
            L.transpose(2, 0, 1, 3).reshape(P, TG * BG * P)
        )
        in_maps.append({"bqT": bqT, "Wc": Wc, "biasr": biasr, "Lt": Lbig})
    return in_maps


last_results = None  # exposed for external profiling harnesses


def kernel(query_weights, basic_queries, W_mlp, b_mlp):
    global last_results
    if "nc" not in _cache:
        _cache["nc"] = _build_nc()
    nc = _cache["nc"]

    in_maps = _prep_inputs(query_weights, basic_queries, W_mlp, b_mlp)
    res = run_bass_kernel_spmd(nc, in_maps, core_ids=list(range(NC)))
    last_results = res

    # dq[j, g, (bb,tt), (h,c)] -> (B, TS, D):  b = h*8+bb, t = g*16+tt,
    # d = j*NW+c
    parts = []
    for c in range(NC):
        arr = res.results[c]["dq"].reshape(NQ, TG, 8, 16, BG, NW)
        arr = arr.transpose(4, 2, 1, 3, 0, 5).reshape(B, TS, D)
        parts.append(arr.astype(np.float32))
    dq_full = np.concatenate(parts, axis=1)
    basic_expanded = np.broadcast_to(
        np.ascontiguousarray(basic_queries, dtype=np.float32)[None], (B, T, D)
    )
    return dq_full, basic_expanded


# revision 8
# speedup vs baseline: 1.5063x; 1.0105x over previous
"""Trainium2 Bass kernel for nn_DynamicDictionaryLearning (vq_codebook).

Computation (full shapes):
    query_embed = (basic_queries @ W_mlp + b_mlp).reshape(T, R, D)    # (T, R*D)
    dynamic_queries = einsum('btr,trd->btd', query_weights, query_embed)
    basic_expanded  = broadcast(basic_queries, (B, T, D))

Sharding (8 NeuronCores, one chip):
    Stage 1 (token-MLP expansion) is tensor-sharded over the R*D output
    dim: core r computes qe_r = basic_queries @ W_mlp[:, r*D:(r+1)*D] +
    b_r for ALL tokens, as 512-col k-outer passes (one PSUM bank per
    token-tile accumulation chain).  Each 256-col chunk is AllToAll'd as
    soon as its slice lands in DRAM; stage-2 chunks (weighted sum over R
    as dense PE matmuls against block-diagonal qw tiles) are interleaved
    into the PE stream one pass behind, so the collectives and both DMA
    rings pipeline behind the PE.

    Schedule notes: W loads are pass-column-sliced so the first pass is
    compute-paced rather than blocked on the full 8.4MB W stream; bq
    rides the SP ring, W the ACT ring.  qe->ain stores are split into
    two half-stores (one per ring), issued as soon as their 4 bias-adds
    retire.  q2 gathers are emitted inside the producing pass so they
    never queue behind a later pass's stores.  bq tokens are host-side
    swizzled (p' = tt*8+g per 128-token tile) which makes each gather a
    single flat contiguous DMA.  PSUM->SBUF casts all run on DVE; dq
    stores are merged per token-group pair.  A full-size dependency-free
    warmup collective absorbs CC bring-up and NEFF start skew.  All
    matmul operands and the dq store are bf16 (fp32 PSUM accumulation).

    basic_expanded is a pure broadcast of an input -> host-side view.
"""

import numpy as np
import ml_dtypes

import concourse.bass as bass
import concourse.mybir as mybir
import concourse.tile as tile
from concourse import bacc
from concourse.bass_utils import run_bass_kernel_spmd

# Problem shapes (hardcoded per spec)
D = 2048
T = 1024
R = 8
B = 32
NC = 8
TS = T // NC          # 128 tokens per core (stage-2 ownership)
P = 128
KT = D // P           # 16 contraction tiles
MT = T // P           # 8 token tiles (stage 1)
NQ = 8                # a2a chunks (D-slices)
NW = D // NQ          # 256 cols per chunk
TG = TS // 16         # 8 token groups of 16 (stage 2)
BG = B // 8           # 4 batch groups of 8 (stage 2)
PSW = 1024            # PSUM slot width (2 banks); 4 slots = all 8 banks

F32 = mybir.dt.float32
BF16 = mybir.dt.bfloat16
DT_MM = BF16
NP_MM = ml_dtypes.bfloat16

# stage-1 passes: (col_lo, width).  The last 512 cols split in two so
# chunks 6/7 finish staggered and their AllToAlls don't serialize.
PASSES = [(0, 512), (512, 512), (1024, 512), (1536, 256), (1792, 256)]
# W column groups (ACT-ring load slabs), one per 512-col band
WGRP = [(0, 512), (512, 512), (1024, 512), (1536, 512)]
# s2 chunks emitted after each pass index (PE program order)
S2_AFTER = {1: [0, 1], 2: [2, 3], 3: [4], 4: [5, 6, 7]}

_cache = {}


def _build_nc():
    nc = bacc.Bacc("TRN2", target_bir_lowering=False, num_devices=NC)

    bqT = nc.dram_tensor("bqT", [D, T], DT_MM, kind="ExternalInput")
    Wc = nc.dram_tensor("Wc", [D, D], DT_MM, kind="ExternalInput")
    biasr = nc.dram_tensor("biasr", [P, D], F32, kind="ExternalInput")
    # block-diagonal qw tiles, packed (128, 32*128) for one big-line DMA
    Lt = nc.dram_tensor("Lt", [P, TG * BG * P], DT_MM, kind="ExternalInput")
    # per-(chunk, token-group-pair) contiguous blocks; host reassembles
    dq = nc.dram_tensor("dq", [NQ, TG // 2, P, 2 * BG * NW], BF16,
                        kind="ExternalOutput")

    bqT_t = bqT.rearrange("(kt p) m -> kt p m", p=P)   # (16, 128, 1024)

    with tile.TileContext(nc) as tc:
        with (
            tc.tile_pool(name="bqp", bufs=1) as bqpool,
            tc.tile_pool(name="wp", bufs=1) as wpool,
            tc.tile_pool(name="constp", bufs=1) as cpool,
            tc.tile_pool(name="qep", bufs=2) as qepool,
            tc.tile_pool(name="q2p", bufs=3) as q2pool,
            tc.tile_pool(name="o2p", bufs=3) as o2pool,
            tc.tile_pool(name="psp", bufs=4, space="PSUM") as pspool,
            tc.tile_pool(name="dramp", bufs=1, space="DRAM") as dram,
        ):
            # warmup collective, full chunk size: absorbs CC bring-up +
            # NEFF start skew while the input loads stream
            wdum = dram.tile([T, NW], DT_MM, name="wdum")
            adum = dram.tile([T, NW], DT_MM, name="adum")
            nc.gpsimd.collective_compute(
                "AllToAll",
                mybir.AluOpType.bypass,
                replica_groups=[list(range(NC))],
                ins=[wdum.opt()],
                outs=[adum.opt()],
            )

            # bq k-tiles on SP ring
            bq_tiles = []
            for k in range(KT):
                bt = bqpool.tile([P, T], DT_MM, name=f"bq{k}")
                nc.sync.dma_start(out=bt, in_=bqT_t[k])
                bq_tiles.append(bt)
            # W on ACT ring, sliced by 512-col band so pass 0 is
            # compute-paced: band 0 all k first, then band 1, ...
            w_tiles = {}
            for ci, (lo, w) in enumerate(WGRP):
                for k in range(KT):
                    wt = wpool.tile([P, w], DT_MM, name=f"w{ci}_{k}")
                    nc.scalar.dma_start(
                        out=wt,
                        in_=Wc[k * P:(k + 1) * P, lo:lo + w],
                    )
                    w_tiles[(ci, k)] = wt
                if ci == 1:
                    bias_t = cpool.tile([P, D], F32, name="bias")
                    nc.scalar.dma_start(out=bias_t, in_=biasr[:, :])
                    lbig = cpool.tile([P, TG * BG * P], DT_MM, name="lbig")
                    nc.scalar.dma_start(out=lbig, in_=Lt[:, :])
            l_tiles = {
                (g, h): lbig[:, (g * BG + h) * P:(g * BG + h + 1) * P]
                for g in range(TG)
                for h in range(BG)
            }

            ain = [dram.tile([T, NW], DT_MM, name=f"ain{j}") for j in range(NQ)]
            aout = [dram.tile([T, NW], DT_MM, name=f"aout{j}") for j in range(NQ)]
            q2_tiles = {}

            def a2a(j):
                nc.gpsimd.collective_compute(
                    "AllToAll",
                    mybir.AluOpType.bypass,
                    replica_groups=[list(range(NC))],
                    ins=[ain[j].opt()],
                    outs=[aout[j].opt()],
                )

            def stage1_pass(pi):
                lo, w = PASSES[pi]
                chunks = list(range(lo // NW, (lo + w) // NW))
                ci = lo // 512
                woff = lo - WGRP[ci][0]
                with nc.named_scope(f"s1_p{pi}"):
                    qe = qepool.tile([P, MT * w], DT_MM, name="qe")
                    ps = [pspool.tile([P, PSW], F32, name="ps")
                          for _ in range(4)]

                    def chain(m):
                        # one accumulation chain per 2KB PSUM bank:
                        # start=True clears the whole bank's has_written
                        # bits, so chains must never share a bank
                        return ps[m // 2][:, (m % 2) * 512:(m % 2) * 512 + w]

                    for k in range(KT):
                        for m in range(MT):
                            nc.tensor.matmul(
                                chain(m),
                                bq_tiles[k][:, m * P:(m + 1) * P],
                                w_tiles[(ci, k)][:, woff:woff + w],
                                start=(k == 0),
                                stop=(k == KT - 1),
                            )
                    for m in range(MT):
                        nc.vector.tensor_add(
                            qe[:, m * w:(m + 1) * w],
                            chain(m),
                            bias_t[:, lo:lo + w],
                        )
                    # qe -> ain: two half-stores per chunk, one per HWDGE
                    # ring, each gated only on its own 4 bias-adds
                    qe3 = qe[:, :].rearrange("p (m c) -> p m c", m=MT)
                    for j in chunks:
                        off = j * NW - lo
                        dst = ain[j].rearrange("(m p) c -> p m c", m=MT)
                        nc.sync.dma_start(
                            out=dst[:, 0:MT // 2, :],
                            in_=qe3[:, 0:MT // 2, off:off + NW],
                        )
                        nc.scalar.dma_start(
                            out=dst[:, MT // 2:MT, :],
                            in_=qe3[:, MT // 2:MT, off:off + NW],
                        )
                    for j in chunks:
                        a2a(j)
                    # gathers for this pass's chunks: emitted here so they
                    # never queue behind a later pass's stores.  bq tokens
                    # are host-swizzled so this is one flat contiguous DMA.
                    for j in chunks:
                        q2 = q2pool.tile([P, TG * NW], DT_MM, name="q2")
                        nc.sync.dma_start(out=q2[:, :], in_=aout[j][:, :])
                        q2_tiles[j] = q2

            def stage2(j):
                with nc.named_scope(f"s2_q{j}"):
                    q2 = q2_tiles.pop(j)
                    for gp in range(TG // 2):
                        o2 = o2pool.tile([P, 2 * BG * NW], BF16, name="o2")
                        for gi in range(2):
                            g = gp * 2 + gi
                            # 4 batch-group matmuls -> one 2-bank PSUM
                            # tile -> one wide DVE cast
                            ps2 = pspool.tile([P, PSW], F32, name="ps")
                            for h in range(BG):
                                nc.tensor.matmul(
                                    ps2[:, h * NW:(h + 1) * NW],
                                    l_tiles[(g, h)][:, :],
                                    q2[:, g * NW:(g + 1) * NW],
                                    start=True,
                                    stop=True,
                                )
                            nc.vector.tensor_copy(
                                o2[:, gi * BG * NW:(gi + 1) * BG * NW],
                                ps2[:, :],
                            )
                        nc.sync.dma_start(out=dq[j, gp], in_=o2[:, :])

            for pi in range(len(PASSES)):
                stage1_pass(pi)
                for j in S2_AFTER.get(pi, []):
                    stage2(j)

    nc.finalize()
    return nc


def _prep_inputs(query_weights, basic_queries, W_mlp, b_mlp):
    qw = np.ascontiguousarray(query_weights, dtype=np.float32)
    bq = np.ascontiguousarray(basic_queries, dtype=np.float32)
    W = np.ascontiguousarray(W_mlp, dtype=np.float32)
    b = np.ascontiguousarray(b_mlp, dtype=np.float32)

    # token swizzle: within each 128-token tile, column p' = tt*8+g holds
    # token g*16+tt, so stage-1 output partitions land pre-packed for the
    # stage-2 gather (see stage1_pass/stage2 in _build_nc).
    m_i = np.arange(MT)[:, None, None]
    tt_j = np.arange(16)[None, :, None]
    g_j = np.arange(TG)[None, None, :]
    tok_idx = (m_i * P + g_j * 16 + tt_j).reshape(-1)  # new-col -> old token
    bqT = np.ascontiguousarray(bq.T[:, tok_idx].astype(NP_MM))  # (D, T)

    g_i = np.arange(TG)[:, None, None, None, None]
    h_i = np.arange(BG)[None, :, None, None, None]
    tt_i = np.arange(16)[None, None, :, None, None]
    r_i = np.arange(R)[None, None, None, :, None]
    bb_i = np.arange(8)[None, None, None, None, :]

    in_maps = []
    for c in range(NC):
        Wc = np.ascontiguousarray(W[:, c * D:(c + 1) * D].astype(NP_MM))
        biasr = np.ascontiguousarray(
            np.broadcast_to(b[c * D:(c + 1) * D], (P, D))
        )
        qw_c = qw[:, c * TS:(c + 1) * TS, :]  # (32, 128, 8)
        # K index r*16+tt (r-major), M index bb*16+tt (b-major)
        L = np.zeros((TG, BG, P, P), NP_MM)
        L[g_i, h_i, r_i * 16 + tt_i, bb_i * 16 + tt_i] = \
            qw_c[h_i * 8 + bb_i, g_i * 16 + tt_i, r_i].astype(NP_MM)
        # pack to (128, 32*128): Lbig[p, (g*BG+h)*128 + m] = L[g, h, p, m]
        Lbig = np.ascontiguousarray(
            L.transpose(2, 0, 1, 3).reshape(P, TG * BG * P)
        )
        in_maps.append({"bqT": bqT, "Wc": Wc, "biasr": biasr, "Lt": Lbig})
    return in_maps


last_results = None  # exposed for external profiling harnesses


def kernel(query_weights, basic_queries, W_mlp, b_mlp):
    global last_results
    if "nc" not in _cache:
        _cache["nc"] = _build_nc()
    nc = _cache["nc"]

    in_maps = _prep_inputs(query_weights, basic_queries, W_mlp, b_mlp)
    res = run_bass_kernel_spmd(nc, in_maps, core_ids=list(range(NC)))
    last_results = res

    # dq[j, gp, (bb,tt), (gi,h,c)] -> (B, TS, D):  b = h*8+bb,
    # t = (2*gp+gi)*16+tt, d = j*NW+c
    parts = []
    for c in range(NC):
        arr = res.results[c]["dq"].reshape(NQ, TG // 2, 8, 16, 2, BG, NW)
        arr = arr.transpose(5, 2, 1, 4, 3, 0, 6).reshape(B, TS, D)
        parts.append(arr.astype(np.float32))
    dq_full = np.concatenate(parts, axis=1)
    basic_expanded = np.broadcast_to(
        np.ascontiguousarray(basic_queries, dtype=np.float32)[None], (B, T, D)
    )
    return dq_full, basic_expanded
